# revision 1
# baseline (speedup 1.0000x reference)
"""Trainium2 Bass kernel for 16-head MultiHeadAttention (B=2, T=2048, D=1024).

Sharding (8 NeuronCores): core c handles batch b = c//4 and head group
g = c%4 (heads 4g..4g+3).  Each core computes Q/K/V projections for its 4
heads, attention, and a partial output projection against its 256 rows of
W_O.  The host sums the 4 partials per batch and adds b_O (row-parallel TP;
the all-reduce is folded into the unshard step).

Device layout notes:
 - The host pre-transposes x to x^T [D, T] so the contraction dim (features)
   lands on SBUF partitions without any on-device transposes of x.
 - Attention is computed in the S^T = K @ Q^T orientation: the softmax
   denominator is then a partition-axis sum, which the PE produces for free
   via a ones-column appended to V (out = [V|1]^T @ P^T gives O^T rows 0..63
   and the denominator in row 64).
 - Per head pair (2 heads of 64), weights are stacked to fill 128 partitions.
 - Matmul operands are bf16 (fp32 PSUM accumulation); softmax denominators,
   reciprocals and the broadcast matmul stay fp32.
 - t-tiles are 1024 wide: bf16 moving operands allow N=1024, and the wide
   EXP activations amortize the ~240ns per-instruction ACT overhead.
"""

import os
import sys

import numpy as np

for _p in ("/opt/trn_rl_repo", "/root/.axon_site/_ro/trn_rl_repo"):
    if os.path.isdir(_p) and _p not in sys.path:
        sys.path.insert(0, _p)

import concourse.bass as bass
import concourse.mybir as mybir
import concourse.tile as tile
from concourse import bacc
from concourse.bass_utils import run_bass_kernel_spmd
from concourse.masks import make_identity

F32 = mybir.dt.float32
BF16 = mybir.dt.bfloat16
AF = mybir.ActivationFunctionType

B, TQ, TK = 2, 2048, 2048
D = 1024          # model dim == x_to/x_from feature dim
H, DH = 16, 64
N_CORES = 8
HEADS_PER_CORE = 4   # one batch per core
HP = 2               # head pairs per core (2 heads of 64 stacked -> 128)

TT = 1024            # t-tile (bf16 moving free dim max)
N_TT = TQ // TT      # 2
N_SC = TK // 128     # 16 s-chunks
N_FC = D // 128      # 8 f-chunks

DT = BF16

_CACHED = {}


def build_program():
    nc = bacc.Bacc(
        "TRN2", target_bir_lowering=False, debug=False, num_devices=N_CORES
    )

    xt_to = nc.dram_tensor("xt_to", [D, TQ], DT, kind="ExternalInput")
    xt_from = nc.dram_tensor("xt_from", [D, TK], DT, kind="ExternalInput")
    wq = nc.dram_tensor("wq", [D, 256], DT, kind="ExternalInput")
    wk = nc.dram_tensor("wk", [D, 256], DT, kind="ExternalInput")
    wv = nc.dram_tensor("wv", [D, 256], DT, kind="ExternalInput")
    bq = nc.dram_tensor("bq", [128, 2], F32, kind="ExternalInput")
    bk = nc.dram_tensor("bk", [128, 2], F32, kind="ExternalInput")
    bv = nc.dram_tensor("bv", [128, 2], F32, kind="ExternalInput")
    wot = nc.dram_tensor("wot", [128, 2, 1024], DT, kind="ExternalInput")
    out = nc.dram_tensor("out", [TQ, D], F32, kind="ExternalOutput")

    with tile.TileContext(nc) as tc:
        with (
            tc.tile_pool(name="wpool", bufs=1) as wpool,
            tc.tile_pool(name="actpool", bufs=1) as actpool,
            tc.tile_pool(name="ptpool", bufs=3) as ptpool,
            tc.tile_pool(name="misc", bufs=2) as misc,
            tc.tile_pool(name="psmm", bufs=2, space="PSUM") as psmm,
            tc.tile_pool(name="psacc", bufs=1, space="PSUM") as psacc,
            tc.tile_pool(name="psaux", bufs=1, space="PSUM") as psaux,
        ):
            # ---- constants & weights -------------------------------------
            ident = wpool.tile([128, 128], DT)
            make_identity(nc, ident[:])

            wq_sb = wpool.tile([128, N_FC, 256], DT)
            wk_sb = wpool.tile([128, N_FC, 256], DT)
            wv_sb = wpool.tile([128, N_FC, 256], DT)
            nc.sync.dma_start(wk_sb[:], wk.rearrange("(c p) d -> p c d", p=128))
            nc.sync.dma_start(wv_sb[:], wv.rearrange("(c p) d -> p c d", p=128))
            nc.sync.dma_start(wq_sb[:], wq.rearrange("(c p) d -> p c d", p=128))

            bq_sb = wpool.tile([128, 2], F32)
            bk_sb = wpool.tile([128, 2], F32)
            bv_sb = wpool.tile([128, 2], F32)
            nc.sync.dma_start(bk_sb[:], bk[:])
            nc.sync.dma_start(bv_sb[:], bv[:])
            nc.sync.dma_start(bq_sb[:], bq[:])

            wot_sb = wpool.tile([128, 2, 1024], DT)
            nc.sync.dma_start(wot_sb[:], wot[:])

            # ---- persistent activations ----------------------------------
            qt_sb = [
                actpool.tile([128, TQ], DT, name=f"qt{hp}") for hp in range(HP)
            ]
            kt_sb = [
                actpool.tile([128, TK], DT, name=f"kt{hp}") for hp in range(HP)
            ]
            vn_sb = [
                actpool.tile([128, N_SC, 130], DT, name=f"vn{hp}")
                for hp in range(HP)
            ]
            ot_sb = [
                actpool.tile([128, TQ], DT, name=f"ot{hp}") for hp in range(HP)
            ]
            # softmax denominators on partition 0: (hp, h) at offset
            # (2*hp+h)*TQ; reciprocal'd in place per slice, bf16 copy feeds
            # the GpSimd partition broadcast
            rec_all = actpool.tile([1, 4 * TQ], F32, name="rec_all")
            den_bf = actpool.tile([1, 4 * TQ], DT, name="den_bf")

            # x^T resident in SBUF, per-f-chunk DMAs, both tensors
            # interleaved so K/V (x_from) and Q (x_to) streams land together
            xfr_sb = actpool.tile([128, N_FC, TK], DT, name="xfr_sb")
            xto_sb = actpool.tile([128, N_FC, TQ], DT, name="xto_sb")
            xt_to_r = xt_to.rearrange("(c p) t -> p c t", p=128)
            xt_from_r = xt_from.rearrange("(c p) t -> p c t", p=128)
            for fc in range(N_FC):
                nc.sync.dma_start(xfr_sb[:, fc, :], xt_from_r[:, fc, :])
                nc.sync.dma_start(xto_sb[:, fc, :], xt_to_r[:, fc, :])

            def emit_proj(ps_name, w_sb, b_sb, x_sb, dst, hp, tt, pool, tag):
                """Return list of thunks: 8 fill steps + 1 copyback."""
                dsl = bass.ts(hp, 128)
                ps = pool.tile([128, TT], F32, tag=tag, name=ps_name)
                thunks = []
                for half in range(2):
                    for fcp in range(N_FC // 2):
                        def fill(half=half, fcp=fcp, ps=ps):
                            for fc in (2 * fcp, 2 * fcp + 1):
                                nc.tensor.matmul(
                                    ps[:, bass.ts(half, 512)],
                                    w_sb[:, fc, dsl],
                                    x_sb[
                                        :, fc,
                                        tt * TT + half * 512 : tt * TT
                                        + half * 512
                                        + 512,
                                    ],
                                    start=(fc == 0),
                                    stop=(fc == N_FC - 1),
                                )
                        thunks.append(fill)

                def copyback(ps=ps):
                    if dst is not None:
                        nc.vector.tensor_scalar_add(
                            dst[hp][:, bass.ts(tt, TT)],
                            ps[:],
                            b_sb[:, hp : hp + 1],
                        )
                    else:
                        vtt = misc.tile([128, TT], DT, tag="vtt", name="vtt")
                        nc.vector.tensor_scalar_add(
                            vtt[:], ps[:], b_sb[:, hp : hp + 1]
                        )
                        copyback.vtt = vtt
                thunks.append(copyback)
                return thunks, copyback

            def emit_kv(hp, tt, pool, tag):
                thunks, _ = emit_proj(
                    "ps_k", wk_sb, bk_sb, xfr_sb, kt_sb, hp, tt, pool, tag
                )
                vthunks, vcb = emit_proj(
                    "ps_v", wv_sb, bv_sb, xfr_sb, None, hp, tt, pool, tag
                )
                thunks += vthunks
                for j in range(TT // 128):
                    def transp(j=j, vcb=vcb):
                        sc = tt * (TT // 128) + j
                        ps_t = pool.tile([128, 128], DT, tag=tag, name="ps_t")
                        nc.tensor.transpose(
                            ps_t[:], vcb.vtt[:, bass.ts(j, 128)], ident[:]
                        )
                        nc.scalar.activation(
                            vn_sb[hp][:, sc, 0:64], ps_t[:, 0:64], AF.Copy
                        )
                        nc.scalar.activation(
                            vn_sb[hp][:, sc, 65:129], ps_t[:, 64:128], AF.Copy
                        )
                    thunks.append(transp)
                return thunks

            def emit_q(hp, tt, pool, tag):
                thunks, _ = emit_proj(
                    "ps_q", wq_sb, bq_sb, xto_sb, qt_sb, hp, tt, pool, tag
                )
                return thunks

            def emit_vn_ones(hp):
                nc.vector.memset(vn_sb[hp][:, :, 64], 1.0)
                nc.vector.memset(vn_sb[hp][:, :, 129], 1.0)

            def emit_stripe(tta, hp, fillers, per_iter=2):
                """One attention stripe: both heads, 512 queries, all keys.
                Pops up to `per_iter` filler thunks per s-chunk so
                independent PE work interleaves finely with the ACT-bound
                exp stream."""
                ps_o = psacc.tile([65, 1024], F32, tag="acc", name="ps_o")
                for sc in range(N_SC):
                    ps_s = psmm.tile([128, 1024], F32, tag="mm", name="ps_s")
                    for h in range(2):
                        hb = 64 * h
                        nc.tensor.matmul(
                            ps_s[:, bass.ts(h, TA)],
                            kt_sb[hp][hb : hb + 64, bass.ts(sc, 128)],
                            qt_sb[hp][hb : hb + 64, bass.ts(tta, TA)],
                            start=True,
                            stop=True,
                        )
                    pt = ptpool.tile([128, 1024], DT, tag="pt", name="pt")
                    nc.scalar.activation(pt[:], ps_s[:], AF.Exp)
                    for h in range(2):
                        vb = 65 * h
                        nc.tensor.matmul(
                            ps_o[:, bass.ts(h, TA)],
                            vn_sb[hp][:, sc, vb : vb + 65],
                            pt[:, bass.ts(h, TA)],
                            start=(sc == 0),
                            stop=(sc == N_SC - 1),
                        )
                    for _ in range(per_iter):
                        if fillers:
                            fillers.popleft()()
                # drain ps_o with the four quick copies FIRST so the single
                # acc slot frees fast; the slow single-lane reciprocals run
                # after and no longer gate the next stripe's PV matmuls
                for h in range(2):
                    hb = 64 * h
                    nc.vector.tensor_copy(
                        ot_sb[hp][hb : hb + 64, bass.ts(tta, TA)],
                        ps_o[0:64, bass.ts(h, TA)],
                    )
                    off = (2 * hp + h) * TQ + tta * TA
                    nc.vector.tensor_copy(
                        rec_all[0:1, off : off + TA],
                        ps_o[64:65, bass.ts(h, TA)],
                    )
                for h in range(2):
                    off = (2 * hp + h) * TQ + tta * TA
                    nc.vector.reciprocal(
                        rec_all[0:1, off : off + TA],
                        rec_all[0:1, off : off + TA],
                    )
                    nc.vector.tensor_copy(
                        den_bf[0:1, off : off + TA],
                        rec_all[0:1, off : off + TA],
                    )
                # normalize: GpSimd broadcasts the reciprocal row across
                # partitions (psum-free), DVE multiplies in place
                for h in range(2):
                    off = (2 * hp + h) * TQ + tta * TA
                    r_sb = misc.tile([128, TA], DT, tag="rsb", name="r_sb")
                    nc.gpsimd.partition_broadcast(
                        r_sb[:], den_bf[0:1, off : off + TA]
                    )
                    hb = 64 * h
                    nc.vector.tensor_mul(
                        ot_sb[hp][hb : hb + 64, bass.ts(tta, TA)],
                        ot_sb[hp][hb : hb + 64, bass.ts(tta, TA)],
                        r_sb[hb : hb + 64, :],
                    )

            def emit_outproj_thunks(tta):
                thunks = []
                for j in range(TA // 128):
                    tc_ = tta * (TA // 128) + j
                    tsl = bass.ts(tc_, 128)
                    ps_out = psmm.tile(
                        [128, 1024], F32, tag="mm", name="ps_out"
                    )
                    for half in range(2):
                        def mmf(half=half, ps_out=ps_out, tsl=tsl):
                            hsl = bass.ts(half, 512)
                            for hp in range(HP):
                                nc.tensor.matmul(
                                    ps_out[:, hsl],
                                    ot_sb[hp][:, tsl],
                                    wot_sb[:, hp, hsl],
                                    start=(hp == 0),
                                    stop=(hp == HP - 1),
                                )
                        thunks.append(mmf)

                    def store(ps_out=ps_out, tc_=tc_, tsl=tsl):
                        o_t = misc.tile(
                            [128, 1024], F32, tag="out", name="o_t"
                        )
                        if tc_ % 2 == 0:
                            nc.vector.tensor_copy(o_t[:], ps_out[:])
                        else:
                            nc.scalar.activation(o_t[:], ps_out[:], AF.Copy)
                        nc.sync.dma_start(out[tsl, :], o_t[:])
                    thunks.append(store)
                return thunks

            # ---- emission schedule ---------------------------------------
            # Both projection blocks run up front: hp0 on the "mm" slots,
            # hp1 on a dedicated aux slot so the attention score-psum FIFO
            # only queues behind hp0's tiles.  All 8 attention stripes then
            # stream gap-free at the ACT exp roofline; output projection
            # trails on the freed "mm" slots.
            from collections import deque

            TA = 512
            empty = deque()
            for f in emit_kv(0, 0, psmm, "mm"):
                f()
            for f in emit_kv(0, 1, psmm, "mm"):
                f()
            for f in emit_q(0, 0, psmm, "mm"):
                f()
            for f in emit_q(0, 1, psmm, "mm"):
                f()
            emit_vn_ones(0)
            for f in emit_kv(1, 0, psaux, "aux"):
                f()
            for f in emit_kv(1, 1, psaux, "aux"):
                f()
            for f in emit_q(1, 0, psaux, "aux"):
                f()
            for f in emit_q(1, 1, psaux, "aux"):
                f()
            emit_vn_ones(1)

            for hp in range(HP):
                for tta in range(TQ // TA):
                    emit_stripe(tta, hp, empty, per_iter=0)

            for tta in range(TQ // TA):
                for f in emit_outproj_thunks(tta):
                    f()

    nc.compile()
    return nc


def _prep_in_maps(x_to, x_from, Wq, bq, Wk, bk, Wv, bv, Wo):
    scale = 1.0 / np.sqrt(np.float32(DH))
    # [H, D, DH] -> [D, H*DH] with column h*DH+d
    wq_f = np.ascontiguousarray(Wq.transpose(1, 0, 2).reshape(D, H * DH)) * scale
    wk_f = np.ascontiguousarray(Wk.transpose(1, 0, 2).reshape(D, H * DH))
    wv_f = np.ascontiguousarray(Wv.transpose(1, 0, 2).reshape(D, H * DH))
    bq_f = bq.reshape(H * DH) * scale
    bk_f = bk.reshape(H * DH)
    bv_f = bv.reshape(H * DH)

    xt_to = np.ascontiguousarray(x_to.transpose(0, 2, 1))    # [B, D, TQ]
    xt_from = np.ascontiguousarray(x_from.transpose(0, 2, 1))

    def f32(a):
        return np.ascontiguousarray(a, dtype=np.float32)

    import ml_dtypes

    def fdt(a):
        return np.ascontiguousarray(a, dtype=ml_dtypes.bfloat16)

    in_maps = []
    for c in range(N_CORES):
        b, g = divmod(c, HEADS_PER_CORE)
        cs = slice(g * 256, (g + 1) * 256)
        in_maps.append(
            {
                "xt_to": fdt(xt_to[b]),
                "xt_from": fdt(xt_from[b]),
                "wq": fdt(wq_f[:, cs]),
                "wk": fdt(wk_f[:, cs]),
                "wv": fdt(wv_f[:, cs]),
                # [256] -> [2 pairs, 128] -> [128, 2]
                "bq": f32(bq_f[cs].reshape(2, 128).T),
                "bk": f32(bk_f[cs].reshape(2, 128).T),
                "bv": f32(bv_f[cs].reshape(2, 128).T),
                # Wo[:, cs].T = [256, 1024] -> [2, 128, 1024] -> [128, 2, 1024]
                "wot": fdt(
                    np.ascontiguousarray(Wo[:, cs].T)
                    .reshape(2, 128, 1024)
                    .transpose(1, 0, 2)
                ),
            }
        )
    return in_maps


LAST_EXEC_TIME_NS = None
LAST_TRACE = None


def kernel(x_to, x_from, Wq, bq, Wk, bk, Wv, bv, Wo, bo):
    global LAST_EXEC_TIME_NS, LAST_TRACE
    if "nc" not in _CACHED:
        _CACHED["nc"] = build_program()
    nc = _CACHED["nc"]

    in_maps = _prep_in_maps(
        np.asarray(x_to), np.asarray(x_from), np.asarray(Wq), np.asarray(bq),
        np.asarray(Wk), np.asarray(bk), np.asarray(Wv), np.asarray(bv),
        np.asarray(Wo),
    )
    res = run_bass_kernel_spmd(nc, in_maps, list(range(N_CORES)))
    LAST_EXEC_TIME_NS = res.exec_time_ns
    LAST_TRACE = res.instructions_and_trace

    out = np.zeros((B, TQ, D), dtype=np.float32)
    for c in range(N_CORES):
        out[c // HEADS_PER_CORE] += res.results[c]["out"]
    out += np.asarray(bo, dtype=np.float32)
    return out



# revision 2
# speedup vs baseline: 1.2316x; 1.2316x over previous
"""Trainium2 Bass kernel for 16-head MultiHeadAttention (B=2, T=2048, D=1024).

Sharding (8 NeuronCores): core c handles batch b = c//4 and head group
g = c%4 (heads 4g..4g+3).  Each core computes Q/K/V projections for its 4
heads, attention, and a partial output projection against its 256 rows of
W_O.  The host sums the 4 partials per batch and adds b_O (row-parallel TP;
the all-reduce is folded into the unshard step).

Device layout notes:
 - The host pre-transposes x to x^T [D, T] so the contraction dim (features)
   lands on SBUF partitions without any on-device transposes of x.
 - Attention is computed in the S^T = K @ Q^T orientation: the softmax
   denominator is then a partition-axis sum, which the PE produces for free
   via a ones-column appended to V (out = [V|1]^T @ P^T gives O^T rows 0..63
   and the denominator in row 64).
 - V^T is produced directly in [s, dh] orientation by swapping matmul
   operand roles (stationary = x_from^T chunk, moving = Wv), with the bias
   AND the ones-columns injected by one extra K=1 matmul against an
   augmented bias row.  No PE transposes, no ACT copies.
 - The scalar engine runs ONLY the exp stream; everything else lives on
   DVE/Pool so ACT stays at its roofline.
 - Softmax reciprocal uses the fast custom-DVE approx (~5x faster than the
   table-based InstReciprocal) on the [1, 512] denominator rows.
 - Projections and the output projection are interleaved into the attention
   stripes as filler thunks so the PE never idles and stays at high pstate.
"""

import os
import sys

from collections import deque

import numpy as np

for _p in ("/opt/trn_rl_repo", "/root/.axon_site/_ro/trn_rl_repo"):
    if os.path.isdir(_p) and _p not in sys.path:
        sys.path.insert(0, _p)

import concourse.bass as bass
import concourse.mybir as mybir
import concourse.tile as tile
from concourse import bacc
from concourse.bass_utils import run_bass_kernel_spmd

F32 = mybir.dt.float32
BF16 = mybir.dt.bfloat16
AF = mybir.ActivationFunctionType

B, TQ, TK = 2, 2048, 2048
D = 1024          # model dim == x_to/x_from feature dim
H, DH = 16, 64
N_CORES = 8
HEADS_PER_CORE = 4   # one batch per core
HP = 2               # head pairs per core (2 heads of 64 stacked -> 128)

TA = 512             # stripe width (queries per stripe)
N_SC = TK // 128     # 16 s-chunks
N_FC = D // 128      # 8 f-chunks
N_TT = 2             # q/k tiles of 1024 along t

DT = BF16

_CACHED = {}


def build_program():
    nc = bacc.Bacc(
        "TRN2", target_bir_lowering=False, debug=False, num_devices=N_CORES
    )

    xt_to = nc.dram_tensor("xt_to", [D, TQ], DT, kind="ExternalInput")
    xt_from = nc.dram_tensor("xt_from", [D, TK], DT, kind="ExternalInput")
    wq = nc.dram_tensor("wq", [D, 256], DT, kind="ExternalInput")
    wk = nc.dram_tensor("wk", [D, 256], DT, kind="ExternalInput")
    wv = nc.dram_tensor("wv", [D, 260], DT, kind="ExternalInput")
    bq = nc.dram_tensor("bq", [128, 2], F32, kind="ExternalInput")
    bk = nc.dram_tensor("bk", [128, 2], F32, kind="ExternalInput")
    bv = nc.dram_tensor("bv", [1, 260], DT, kind="ExternalInput")
    wot = nc.dram_tensor("wot", [128, 2, 1024], DT, kind="ExternalInput")
    out = nc.dram_tensor("out", [TQ, D], F32, kind="ExternalOutput")

    with tile.TileContext(nc) as tc:
        with (
            tc.tile_pool(name="wpool", bufs=1) as wpool,
            tc.tile_pool(name="actpool", bufs=1) as actpool,
            tc.tile_pool(name="ptpool", bufs=3) as ptpool,
            tc.tile_pool(name="misc", bufs=2) as misc,
            tc.tile_pool(name="psmm", bufs=2, space="PSUM") as psmm,
            tc.tile_pool(name="psacc", bufs=1, space="PSUM") as psacc,
            tc.tile_pool(name="psaux", bufs=2, space="PSUM") as psaux,
        ):
            # ---- weights / constants -------------------------------------
            wq_sb = wpool.tile([128, N_FC, 256], DT)
            wk_sb = wpool.tile([128, N_FC, 256], DT)
            wv_sb = wpool.tile([128, N_FC, 260], DT)
            bq_sb = wpool.tile([128, 2], F32)
            bk_sb = wpool.tile([128, 2], F32)
            bv_sb = wpool.tile([1, 260], DT)
            wot_sb = wpool.tile([128, 2, 1024], DT)
            ones_sb = wpool.tile([1, 128], DT)
            nc.vector.memset(ones_sb[:], 1.0)

            # x^T resident in SBUF, per-f-chunk DMAs
            xfr_sb = actpool.tile([128, N_FC, TK], DT, name="xfr_sb")
            xto_sb = actpool.tile([128, N_FC, TQ], DT, name="xto_sb")
            xt_to_r = xt_to.rearrange("(c p) t -> p c t", p=128)
            xt_from_r = xt_from.rearrange("(c p) t -> p c t", p=128)

            # DMA priority: K/V inputs and weights first so the prologue
            # projections can start as soon as possible.
            nc.sync.dma_start(wk_sb[:], wk.rearrange("(c p) d -> p c d", p=128))
            nc.sync.dma_start(wv_sb[:], wv.rearrange("(c p) d -> p c d", p=128))
            nc.sync.dma_start(bv_sb[:], bv[:])
            nc.sync.dma_start(bk_sb[:], bk[:])
            for fc in range(N_FC):
                nc.sync.dma_start(xfr_sb[:, fc, :], xt_from_r[:, fc, :])
            nc.sync.dma_start(wq_sb[:], wq.rearrange("(c p) d -> p c d", p=128))
            nc.sync.dma_start(bq_sb[:], bq[:])
            for fc in range(N_FC):
                nc.sync.dma_start(xto_sb[:, fc, :], xt_to_r[:, fc, :])
            nc.sync.dma_start(wot_sb[:], wot[:])

            # ---- persistent activations ----------------------------------
            qt_sb = [
                actpool.tile([128, TQ], DT, name=f"qt{hp}") for hp in range(HP)
            ]
            kt_sb = [
                actpool.tile([128, TK], DT, name=f"kt{hp}") for hp in range(HP)
            ]
            # V^T with ones columns: head h at cols 65h..65h+63, ones at
            # 65h+64 (4 heads -> 260 cols), per 128-wide s-chunk
            vn_sb = actpool.tile([128, N_SC, 260], DT, name="vn_sb")
            ot_sb = [
                actpool.tile([128, TQ], DT, name=f"ot{hp}") for hp in range(HP)
            ]

            # ---- thunk emitters ------------------------------------------
            def qk_thunks(w_sb, b_sb, x_sb, dst, hp, tt):
                """Q/K projection for one [128, 1024] tile: two psum halves,
                each 8 accumulating matmuls + a bias-add copyback."""
                thunks = []
                dsl = bass.ts(hp, 128)
                for half in range(2):
                    ps = psaux.tile([128, 512], F32, name="ps_x")
                    t0 = tt * 1024 + half * 512
                    for fcp in range(N_FC // 2):
                        def fill(fcp=fcp, ps=ps, t0=t0):
                            for fc in (2 * fcp, 2 * fcp + 1):
                                nc.tensor.matmul(
                                    ps[:],
                                    w_sb[:, fc, dsl],
                                    x_sb[:, fc, t0:t0 + 512],
                                    start=(fc == 0),
                                    stop=(fc == N_FC - 1),
                                )
                        thunks.append(fill)

                    def copyback(ps=ps, t0=t0):
                        nc.vector.tensor_scalar_add(
                            dst[hp][:, t0:t0 + 512], ps[:], b_sb[:, hp:hp + 1]
                        )
                    thunks.append(copyback)
                return thunks

            def v_thunks(sc):
                """V^T for one s-chunk, computed directly in [s, dh]
                orientation: stationary = x_from^T chunk, moving = Wv.
                Bias + ones columns injected via a K=1 matmul."""
                thunks = []
                ps = psaux.tile([128, 512], F32, name="ps_x")
                ssl = bass.ts(sc, 128)
                for fcp in range(N_FC // 2):
                    def fill(fcp=fcp, ps=ps):
                        for fc in (2 * fcp, 2 * fcp + 1):
                            nc.tensor.matmul(
                                ps[:, 0:260],
                                xfr_sb[:, fc, ssl],
                                wv_sb[:, fc, :],
                                start=(fc == 0),
                                stop=False,
                            )
                    thunks.append(fill)

                def bias(ps=ps):
                    nc.tensor.matmul(
                        ps[:, 0:260],
                        ones_sb[:],
                        bv_sb[:],
                        start=False,
                        stop=True,
                    )
                thunks.append(bias)

                def copyback(ps=ps):
                    nc.vector.tensor_copy(vn_sb[:, sc, :], ps[:, 0:260])
                thunks.append(copyback)
                return thunks

            def outproj_thunks(tta):
                """Output projection for one stripe of queries: 4 t-chunks
                of 128, each = 2 psum halves (contraction over both head
                pairs) + copyback, then one DMA."""
                thunks = []
                for j in range(TA // 128):
                    tc_ = tta * (TA // 128) + j
                    tsl = bass.ts(tc_, 128)
                    o_t = misc.tile([128, 1024], F32, name="o_t")
                    for half in range(2):
                        ps = psaux.tile([128, 512], F32, name="ps_x")
                        hsl = bass.ts(half, 512)

                        def mmf(ps=ps, tsl=tsl, hsl=hsl):
                            for hp in range(HP):
                                nc.tensor.matmul(
                                    ps[:],
                                    ot_sb[hp][:, tsl],
                                    wot_sb[:, hp, hsl],
                                    start=(hp == 0),
                                    stop=(hp == HP - 1),
                                )
                        thunks.append(mmf)

                        def cb(ps=ps, o_t=o_t, hsl=hsl):
                            nc.vector.tensor_copy(o_t[:, hsl], ps[:])
                        thunks.append(cb)

                    def store(o_t=o_t, tsl=tsl):
                        nc.sync.dma_start(out[tsl, :], o_t[:])
                    thunks.append(store)
                return thunks

            def emit_stripe(tta, hp, fillers, per_iter):
                """One attention stripe: both heads of the pair, 512
                queries, all 2048 keys.  Pops filler thunks per s-chunk so
                independent PE work interleaves with the ACT exp stream."""
                ps_o = psacc.tile([65, 1024], F32, name="ps_o")
                for sc in range(N_SC):
                    ps_s = psmm.tile([128, 1024], F32, name="ps_s")
                    for h in range(2):
                        hb = 64 * h
                        nc.tensor.matmul(
                            ps_s[:, bass.ts(h, TA)],
                            kt_sb[hp][hb:hb + 64, bass.ts(sc, 128)],
                            qt_sb[hp][hb:hb + 64, bass.ts(tta, TA)],
                            start=True,
                            stop=True,
                        )
                    pt = ptpool.tile([128, 1024], DT, name="pt")
                    nc.scalar.activation(pt[:], ps_s[:], AF.Exp)
                    for h in range(2):
                        vb = 65 * (2 * hp + h)
                        nc.tensor.matmul(
                            ps_o[:, bass.ts(h, TA)],
                            vn_sb[:, sc, vb:vb + 65],
                            pt[:, bass.ts(h, TA)],
                            start=(sc == 0),
                            stop=(sc == N_SC - 1),
                        )
                    for _ in range(per_iter):
                        if fillers:
                            fillers.popleft()()

                # denominators (psum row 64) -> fast reciprocal -> broadcast
                # across partitions (Pool) -> normalize ps_o into ot (DVE)
                recs = []
                for h in range(2):
                    rec = misc.tile([1, TA], F32, name="rec_t")
                    nc.vector.tensor_copy(rec[:], ps_o[64:65, bass.ts(h, TA)])
                    nc.vector.reciprocal_approx_fast(rec[:], rec[:])
                    recs.append(rec)
                for h in range(2):
                    r_sb = misc.tile([128, TA], F32, name="r_sb")
                    nc.gpsimd.partition_broadcast(r_sb[:], recs[h][:])
                    hb = 64 * h
                    nc.vector.tensor_mul(
                        ot_sb[hp][hb:hb + 64, bass.ts(tta, TA)],
                        ps_o[0:64, bass.ts(h, TA)],
                        r_sb[0:64, :],
                    )

            # ---- emission schedule ---------------------------------------
            # Prologue: K(hp0), Q(hp0, tt0) and all of V so the first
            # stripe can start; everything else becomes filler inside the
            # stripes.  Stripes run tta-major so the output projection of
            # stripe tta can fill stripes tta+1...
            for f in qk_thunks(wk_sb, bk_sb, xfr_sb, kt_sb, 0, 0):
                f()
            for f in qk_thunks(wk_sb, bk_sb, xfr_sb, kt_sb, 0, 1):
                f()
            for sc in range(N_SC):
                for f in v_thunks(sc):
                    f()
            for f in qk_thunks(wq_sb, bq_sb, xto_sb, qt_sb, 0, 0):
                f()

            fillers = deque()
            fillers.extend(qk_thunks(wk_sb, bk_sb, xfr_sb, kt_sb, 1, 0))
            fillers.extend(qk_thunks(wk_sb, bk_sb, xfr_sb, kt_sb, 1, 1))
            fillers.extend(qk_thunks(wq_sb, bq_sb, xto_sb, qt_sb, 1, 0))
            fillers.extend(qk_thunks(wq_sb, bq_sb, xto_sb, qt_sb, 0, 1))
            fillers.extend(qk_thunks(wq_sb, bq_sb, xto_sb, qt_sb, 1, 1))

            for tta in range(TQ // TA):
                for hp in range(HP):
                    per_iter = 2 if tta < 2 else 1
                    emit_stripe(tta, hp, fillers, per_iter)
                fillers.extend(outproj_thunks(tta))

            while fillers:
                fillers.popleft()()

    nc.compile()
    return nc


def _prep_in_maps(x_to, x_from, Wq, bq, Wk, bk, Wv, bv, Wo):
    scale = 1.0 / np.sqrt(np.float32(DH))
    # [H, D, DH] -> [D, H*DH] with column h*DH+d
    wq_f = np.ascontiguousarray(Wq.transpose(1, 0, 2).reshape(D, H * DH)) * scale
    wk_f = np.ascontiguousarray(Wk.transpose(1, 0, 2).reshape(D, H * DH))
    bq_f = bq.reshape(H * DH) * scale
    bk_f = bk.reshape(H * DH)

    xt_to = np.ascontiguousarray(x_to.transpose(0, 2, 1))    # [B, D, TQ]
    xt_from = np.ascontiguousarray(x_from.transpose(0, 2, 1))

    def f32(a):
        return np.ascontiguousarray(a, dtype=np.float32)

    import ml_dtypes

    def fdt(a):
        return np.ascontiguousarray(a, dtype=ml_dtypes.bfloat16)

    in_maps = []
    for c in range(N_CORES):
        b, g = divmod(c, HEADS_PER_CORE)
        cs = slice(g * 256, (g + 1) * 256)
        # Wv augmented: head h (of the core's 4) at cols 65h..65h+63,
        # zero col at 65h+64; bias row gets bv there plus 1.0 ones
        wv_aug = np.zeros((D, 260), dtype=np.float32)
        bv_aug = np.zeros((260,), dtype=np.float32)
        for h in range(4):
            head = 4 * g + h
            wv_aug[:, 65 * h:65 * h + 64] = Wv[head]
            bv_aug[65 * h:65 * h + 64] = bv[head]
            bv_aug[65 * h + 64] = 1.0
        in_maps.append(
            {
                "xt_to": fdt(xt_to[b]),
                "xt_from": fdt(xt_from[b]),
                "wq": fdt(wq_f[:, cs]),
                "wk": fdt(wk_f[:, cs]),
                "wv": fdt(wv_aug),
                # [256] -> [2 pairs, 128] -> [128, 2]
                "bq": f32(bq_f[cs].reshape(2, 128).T),
                "bk": f32(bk_f[cs].reshape(2, 128).T),
                "bv": fdt(bv_aug.reshape(1, 260)),
                # Wo[:, cs].T = [256, 1024] -> [2, 128, 1024] -> [128, 2, 1024]
                "wot": fdt(
                    np.ascontiguousarray(Wo[:, cs].T)
                    .reshape(2, 128, 1024)
                    .transpose(1, 0, 2)
                ),
            }
        )
    return in_maps


LAST_EXEC_TIME_NS = None
LAST_TRACE = None


def kernel(x_to, x_from, Wq, bq, Wk, bk, Wv, bv, Wo, bo):
    global LAST_EXEC_TIME_NS, LAST_TRACE
    if "nc" not in _CACHED:
        _CACHED["nc"] = build_program()
    nc = _CACHED["nc"]

    in_maps = _prep_in_maps(
        np.asarray(x_to), np.asarray(x_from), np.asarray(Wq), np.asarray(bq),
        np.asarray(Wk), np.asarray(bk), np.asarray(Wv), np.asarray(bv),
        np.asarray(Wo),
    )
    res = run_bass_kernel_spmd(nc, in_maps, list(range(N_CORES)))
    LAST_EXEC_TIME_NS = res.exec_time_ns
    LAST_TRACE = res.instructions_and_trace

    out = np.zeros((B, TQ, D), dtype=np.float32)
    for c in range(N_CORES):
        out[c // HEADS_PER_CORE] += res.results[c]["out"]
    out += np.asarray(bo, dtype=np.float32)
    return out
